# revision 1
# baseline (speedup 1.0000x reference)
"""Trainium2 Bass kernel for nn_CircuitModel (soft sequential XOR circuit).

Math: with u = 1 - 2*s (s = register value), soft-XOR becomes a pure product:
    u_new = u_a * u_b,   u_x = -tanh(2*clip(p, -2, 2))
Magnitudes and signs decouple and both evolve *linearly*, so the whole
64-step scan unrolls at build time (wa/wb known when kernel() is called):
    log|u_final[j]| = sum_k C[j,k] * ln(tanh(2|p_k|))   (k = used (i,t) cell)
    sign(u_final[j]) = (-1)^( sum_k C[j,k] * [p_k > 0]  mod 2 )
    (parity of C.v == parity of D.v for D = C mod 2)
Registers whose tree hits the t=0 init state (u=0) or has leaf count
>= 4096 (magnitude underflow) output exactly 0.5.

Only ~209 of 256 registers are nontrivial, with ~703 used (i,t) cells.
Sharding: pure batch-parallel, 512 batch per core. The host packs the used
cells in sign-magnitude form (a pure re-layout of the same numbers):
  ap = |p| as bf16 [128, nchunks, 512],  v = [p > 0] as fp8 {0,1}.

Device per core / rep:
  phase A (ACT table exp_and_others):
    t = tanh(2*ap) [ACT, >= 0 so ln needs no abs; no exact zeros in bf16]
    Y = D^T v: fp8 DoubleRow matmuls (2 chunks/pass, 4x bf16 throughput;
        0/1 counts < 4096 exact in f32 PSUM)
    parity of Y via round-to-nearest magic (DVE):
        g = Y/2 + 2^23; h = g - 2^23 (= RN(Y/2)); d = Y/2 - h in {0,+-1/2}
        b = [d != 0] - 1/2  in {+-1/2} (bf16 exact)
  phase B (ACT table natural_log_exp_and_others):
    l = ln(t) [ACT]; L = C^T l: bf16 matmuls
    e = exp(L) [ACT]; s = b*e + 1/2 [DVE] -> bf16 out, host casts to f32
Reps are grouped G=6 per ACT table phase so the steady state pays the two
1283ns table loads once per group; PSUM stays at 8 banks (Y/L tiles are
double-buffered and die within their phase).
"""

import sys
from contextlib import ExitStack

import numpy as np

sys.path.insert(0, "/opt/trn_rl_repo")

import concourse.mybir as mybir  # noqa: E402
import concourse.tile as tile  # noqa: E402
from concourse import bacc, bass_utils  # noqa: E402

N_IN = 256
N_REG = 256
T = 64
B = 4096
NCORES = 8
BL = B // NCORES  # 512 batch per core
W_CAP = 4096  # leaf-count threshold beyond which u underflows to 0 -> s = 0.5

AF = mybir.ActivationFunctionType
ALU = mybir.AluOpType
MAGIC = float(1 << 23)  # 2^23: fp32 round-to-nearest-integer magic


def _unroll(wa, wb):
    """Exact symbolic unroll of the 64-step recurrence.

    Returns (C counts int64 [N_REG, N_IN*T] saturating, Z bool: u == 0
    exactly because the tree reaches the init state)."""
    NC = N_IN * T
    C = np.zeros((N_REG, NC), np.int64)
    Z = np.ones(N_REG, bool)
    wa = np.asarray(wa).astype(np.int64)
    wb = np.asarray(wb).astype(np.int64)
    for t in range(T):
        nC = np.zeros_like(C)
        nZ = np.zeros(N_REG, bool)
        for src in (wa, wb):
            is_x = src < N_IN
            xrows = np.nonzero(is_x)[0]
            nC[xrows, src[xrows] * T + t] += 1
            rrows = np.nonzero(~is_x)[0]
            ri = src[rrows] - N_IN
            nC[rrows] += C[ri]
            nZ[rrows] |= Z[ri]
        np.minimum(nC, 1 << 20, out=nC)
        C, Z = nC, nZ
    return C, Z


def _build_plan(wa, wb):
    C, Z = _unroll(wa, wb)
    W = C.sum(1)
    alive = (~Z) & (W < W_CAP)
    aidx = np.nonzero(alive)[0]
    A = int(len(aidx))
    if A == 0:
        return {"A": 0, "aidx": aidx}
    Ca = C[aidx]
    used = (Ca != 0).any(0)
    cols = np.nonzero(used)[0]  # flattened (i*T + t) indices of used cells
    ncols = int(len(cols))
    nchunks = (ncols + 127) // 128
    nslots = nchunks * 128
    # pad slots duplicate the first used cell; their C columns stay zero
    slot_cols = np.concatenate([cols, np.full(nslots - ncols, cols[0], np.int64)])
    slot_i = slot_cols // T
    slot_t = slot_cols % T

    n_mt = (A + 127) // 128
    Apad = n_mt * 128
    # lhsT layout [slot-in-chunk (K), chunk * Apad + alive-row (M)]
    cw = np.zeros((128, nchunks * Apad), np.float32)
    dw = np.zeros((128, nchunks * Apad), np.float32)
    for s in range(ncols):
        ci = slot_cols[s]
        c, k = divmod(s, 128)
        cw[k, c * Apad : c * Apad + A] = Ca[:, ci]
        dw[k, c * Apad : c * Apad + A] = Ca[:, ci] % 2
    return {
        "A": A,
        "aidx": aidx,
        "slot_i": slot_i,
        "slot_t": slot_t,
        "nchunks": nchunks,
        "n_mt": n_mt,
        "Apad": Apad,
        "cw": cw,
        "dw": dw,
    }


def _build_nc(plan, reps=1, loop=1):
    """reps: python-unrolled body repetitions (grouped G per ACT table
    phase). loop: hardware For_i trip count around the body (benchmarking
    steady state via wall clock without instruction blowup)."""
    f32 = mybir.dt.float32
    bf16 = mybir.dt.bfloat16
    f8 = mybir.dt.float8e4
    DR = mybir.MatmulPerfMode.DoubleRow
    nchunks, n_mt, Apad = plan["nchunks"], plan["n_mt"], plan["Apad"]
    G = 6  # reps per ACT-table phase

    nc = bacc.Bacc("TRN2", debug=False)
    # [slot-in-chunk, chunk, batch], packed on host while sharding
    ap_d = nc.dram_tensor("ap_used", [128, nchunks, BL], bf16, kind="ExternalInput")
    v_d = nc.dram_tensor("v_used", [128, nchunks, BL], f8, kind="ExternalInput")
    cw_d = nc.dram_tensor("cw", [128, nchunks * Apad], bf16, kind="ExternalInput")
    dw_d = nc.dram_tensor("dw", [128, nchunks * Apad], f8, kind="ExternalInput")
    out_d = nc.dram_tensor("outs", [Apad, BL], bf16, kind="ExternalOutput")

    with tile.TileContext(nc) as tc, ExitStack() as ctx:
        pool = ctx.enter_context(tc.tile_pool(name="pool", bufs=1))
        tmp = ctx.enter_context(tc.tile_pool(name="tmp", bufs=2))
        mps = ctx.enter_context(tc.tile_pool(name="mps", bufs=1, space="PSUM"))

        cw_s = pool.tile([128, nchunks * Apad], bf16)
        nc.sync.dma_start(cw_s[:], cw_d[:])
        dw_s = pool.tile([128, nchunks * Apad], f8)
        nc.sync.dma_start(dw_s[:], dw_d[:])
        cwv = cw_s.rearrange("k (c a) -> k c a", a=Apad)
        dwv = dw_s.rearrange("k (c a) -> k c a", a=Apad)

        def body():
            for r0 in range(0, reps, G):
                g = min(G, reps - r0)
                refs = []
                # ---- phase A: ACT table exp_and_others (tanh) ----
                for i in range(g):
                    ap_s = pool.tile(
                        [128, nchunks * BL], bf16, name=f"ap{i%2}", tag=f"ap{i%2}"
                    )
                    nc.sync.dma_start(ap_s[:], ap_d.rearrange("k c b -> k (c b)"))
                    v_s = pool.tile(
                        [128, nchunks * BL], f8, name=f"v{i%2}", tag=f"v{i%2}"
                    )
                    nc.sync.dma_start(v_s[:], v_d.rearrange("k c b -> k (c b)"))
                    vv = v_s.rearrange("k (c b) -> k c b", b=BL)
                    t_s = pool.tile(
                        [128, nchunks * BL], bf16, name=f"t{i}", tag=f"t{i}"
                    )
                    nc.scalar.activation(t_s[:], ap_s[:], AF.Tanh, scale=2.0)
                    Y_ps = mps.tile(
                        [128, n_mt * BL], f32, name=f"Yp{i%2}", tag=f"Yp{i%2}"
                    )
                    npair = nchunks // 2
                    for c in range(npair):  # fp8 DoubleRow: 2 chunks per pass
                        for mt in range(n_mt):
                            nc.tensor.matmul(
                                Y_ps[:, mt * BL : (mt + 1) * BL],
                                dwv[:, 2 * c : 2 * c + 2, mt * 128 : (mt + 1) * 128],
                                vv[:, 2 * c : 2 * c + 2, :],
                                start=(c == 0),
                                stop=(c == npair - 1 and nchunks % 2 == 0),
                                perf_mode=DR,
                            )
                    if nchunks % 2:  # odd tail chunk: plain fp8 matmul
                        c = nchunks - 1
                        for mt in range(n_mt):
                            nc.tensor.matmul(
                                Y_ps[:, mt * BL : (mt + 1) * BL],
                                dwv[:, c, mt * 128 : (mt + 1) * 128],
                                vv[:, c, :],
                                start=False,
                                stop=True,
                            )
                    # parity: b = [Y odd] - 1/2, exact via fp32 magic round
                    g_t = tmp.tile([128, n_mt * BL], f32, tag="p_g")
                    nc.vector.tensor_scalar(
                        g_t[:], Y_ps[:], 0.5, MAGIC, ALU.mult, ALU.add
                    )
                    h_t = tmp.tile([128, n_mt * BL], f32, tag="p_h")
                    nc.vector.tensor_scalar(h_t[:], g_t[:], MAGIC, None, ALU.subtract)
                    d_t = tmp.tile([128, n_mt * BL], bf16, tag="p_d")
                    nc.vector.scalar_tensor_tensor(
                        d_t[:], Y_ps[:], 0.5, h_t[:], ALU.mult, ALU.subtract
                    )
                    b_t = pool.tile([128, n_mt * BL], bf16, name=f"b{i}", tag=f"b{i}")
                    nc.vector.tensor_scalar(
                        b_t[:], d_t[:], 0.0, 0.5, ALU.not_equal, ALU.subtract
                    )
                    refs.append((t_s, b_t))
                # ---- phase B: ACT table natural_log_exp_and_others ----
                for i in range(g):
                    t_s, b_t = refs[i]
                    l_s = pool.tile(
                        [128, nchunks * BL], bf16, name=f"l{i%2}", tag=f"l{i%2}"
                    )
                    nc.scalar.activation(l_s[:], t_s[:], AF.Ln)
                    L_ps = mps.tile(
                        [128, n_mt * BL], f32, name=f"Lp{i%2}", tag=f"Lp{i%2}"
                    )
                    lv = l_s.rearrange("k (c b) -> k c b", b=BL)
                    for c in range(nchunks):
                        for mt in range(n_mt):
                            nc.tensor.matmul(
                                L_ps[:, mt * BL : (mt + 1) * BL],
                                cwv[:, c, mt * 128 : (mt + 1) * 128],
                                lv[:, c, :],
                                start=(c == 0),
                                stop=(c == nchunks - 1),
                            )
                    e_t = tmp.tile([128, n_mt * BL], bf16, tag="p_e")
                    nc.scalar.activation(e_t[:], L_ps[:], AF.Exp)
                    y_t = tmp.tile([128, n_mt * BL], bf16, tag="p_y")
                    nc.vector.tensor_tensor(y_t[:], b_t[:], e_t[:], ALU.mult)
                    s_t = tmp.tile([128, n_mt * BL], bf16, tag="p_s")
                    nc.vector.tensor_scalar(s_t[:], y_t[:], 0.5, None, ALU.add)
                    nc.sync.dma_start(
                        out_d.rearrange("(m k) b -> k m b", m=n_mt),
                        s_t.rearrange("k (m b) -> k m b", m=n_mt),
                    )

        if loop > 1:
            with tc.For_i(0, loop):
                body()
        else:
            body()

    nc.compile()
    return nc


_CACHE = {}


def _get_compiled(wa, wb):
    key = (np.asarray(wa).tobytes(), np.asarray(wb).tobytes())
    if key not in _CACHE:
        plan = _build_plan(wa, wb)
        nc = _build_nc(plan) if plan["A"] > 0 else None
        _CACHE[key] = (plan, nc)
    return _CACHE[key]


def _pack_core(P, plan, c):
    """Pack core c's slot tensors: (|p| bf16, [p>0] fp8) [128, nchunks, BL]."""
    bf = mybir.dt.np(mybir.dt.bfloat16)
    f8 = mybir.dt.np(mybir.dt.float8e4)
    sel = P[plan["slot_i"], c * BL : (c + 1) * BL, plan["slot_t"]]  # [nslots, BL]
    nchunks = plan["nchunks"]
    sel = sel.reshape(nchunks, 128, BL).transpose(1, 0, 2)
    ap = np.ascontiguousarray(np.abs(sel)).astype(bf)
    v = np.ascontiguousarray((sel > 0).astype(np.float32)).astype(f8)
    return ap, v


def run(P, wa, wb, trace=False):
    """Returns (out [B, N_REG] float32, BassKernelResults-or-None)."""
    P = np.asarray(P)
    plan, nc = _get_compiled(wa, wb)
    out = np.full((B, N_REG), 0.5, np.float32)
    if plan["A"] == 0:
        return out, None

    bf = mybir.dt.np(mybir.dt.bfloat16)
    f8 = mybir.dt.np(mybir.dt.float8e4)
    cw = plan["cw"].astype(bf)
    dw = plan["dw"].astype(f8)
    in_maps = []
    for c in range(NCORES):
        ap, v = _pack_core(P, plan, c)
        in_maps.append({"ap_used": ap, "v_used": v, "cw": cw, "dw": dw})

    res = bass_utils.run_bass_kernel_spmd(
        nc, in_maps, list(range(NCORES)), trace=trace
    )
    A = plan["A"]
    aidx = plan["aidx"]
    for c in range(NCORES):
        s_core = np.asarray(res.results[c]["outs"]).astype(np.float32)  # [Apad, BL]
        out[c * BL : (c + 1) * BL, aidx] = s_core[:A].T
    return out, res


def kernel(P, wa, wb):
    out, _ = run(P, wa, wb, trace=False)
    return out



# revision 4
# speedup vs baseline: 60.2536x; 60.2536x over previous
"""Trainium2 Bass kernel for nn_CircuitModel (soft sequential XOR circuit).

Math: with u = 1 - 2*s (s = register value), soft-XOR becomes a pure product:
    u_new = u_a * u_b,   u_x = -tanh(2*clip(p, -2, 2))
Magnitudes and signs decouple and both evolve *linearly*, so the whole
64-step scan unrolls at build time (wa/wb known when kernel() is called):
    log|u_final[j]| = sum_k C[j,k] * ln(tanh(2|p_k|))   (k = used (i,t) cell)
    sign(u_final[j]) = (-1)^( sum_k C[j,k] * [p_k > 0]  mod 2 )
    (parity of C.v == parity of D.v for D = C mod 2)
Registers whose tree hits the t=0 init state (u=0) or has leaf count
>= 4096 (magnitude underflow) output exactly 0.5.

Sharding: pure batch-parallel, 512 batch per core. The only per-exec
input is the signed bf16 value of each used (i,t) cell:
    ap = p as bf16 [128, nchunks, 512]   (a pure re-layout of P's numbers)
The wiring matrices cw (counts, bf16) and dw (parity, fp8) are baked into
the NEFF as Const tensors (loaded to device DRAM once at model load, not
per execution). Signs v = [p > 0] are computed on device (GpSimd), |t| on
DVE, so neither is transferred. Output is uint8 round(s*254 + ...) --
quantization error <= 1/508, well inside the 2e-2 gate.

Device per core / rep:
  phase A (ACT table exp_and_others):
    t = tanh(2*ap) [ACT, signed]
    v = [ap > 0] fp8 {0,1} [GpSimd]     ta = abs_max(t, 0) [DVE]
    Y = D^T v: fp8 DoubleRow matmuls (2 chunks/pass; counts exact in f32)
    parity of Y via round-to-nearest magic (DVE):
        g = Y/2 + 2^23; h = g - 2^23 (= RN(Y/2)); d = Y/2 - h in {0,+-1/2}
        b = [d != 0] - 1/2  in {+-1/2} (bf16 exact)
  phase B (ACT table natural_log_exp_and_others):
    l = ln(ta) [ACT]; L = C^T l: bf16 matmuls
    e = exp(L) [ACT]; y = b*e [DVE]; q = uint8(y*254 + 128) [DVE] -> D2H
Host decodes s = (q - 127.75)/254 + 0.5 (split-the-difference for
truncate-vs-round cast semantics; adds <= 1/1016 error).
"""

import sys
from contextlib import ExitStack

import numpy as np

sys.path.insert(0, "/opt/trn_rl_repo")

import concourse.mybir as mybir  # noqa: E402
import concourse.tile as tile  # noqa: E402
from concourse import bacc, bass_utils  # noqa: E402

N_IN = 256
N_REG = 256
T = 64
B = 4096
NCORES = 8
BL = B // NCORES  # 512 batch per core
W_CAP = 4096  # leaf-count threshold beyond which u underflows to 0 -> s = 0.5

AF = mybir.ActivationFunctionType
ALU = mybir.AluOpType
MAGIC = float(1 << 23)  # 2^23: fp32 round-to-nearest-integer magic


def _unroll(wa, wb):
    """Exact symbolic unroll of the 64-step recurrence.

    Returns (C counts int64 [N_REG, N_IN*T] saturating, Z bool: u == 0
    exactly because the tree reaches the init state)."""
    NC = N_IN * T
    C = np.zeros((N_REG, NC), np.int64)
    Z = np.ones(N_REG, bool)
    wa = np.asarray(wa).astype(np.int64)
    wb = np.asarray(wb).astype(np.int64)
    for t in range(T):
        nC = np.zeros_like(C)
        nZ = np.zeros(N_REG, bool)
        for src in (wa, wb):
            is_x = src < N_IN
            xrows = np.nonzero(is_x)[0]
            nC[xrows, src[xrows] * T + t] += 1
            rrows = np.nonzero(~is_x)[0]
            ri = src[rrows] - N_IN
            nC[rrows] += C[ri]
            nZ[rrows] |= Z[ri]
        np.minimum(nC, 1 << 20, out=nC)
        C, Z = nC, nZ
    return C, Z


def _build_plan(wa, wb):
    C, Z = _unroll(wa, wb)
    W = C.sum(1)
    alive = (~Z) & (W < W_CAP)
    aidx = np.nonzero(alive)[0]
    A = int(len(aidx))
    if A == 0:
        return {"A": 0, "aidx": aidx}
    Ca = C[aidx]
    used = (Ca != 0).any(0)
    cols = np.nonzero(used)[0]  # flattened (i*T + t) indices of used cells
    ncols = int(len(cols))
    nchunks = (ncols + 127) // 128
    nslots = nchunks * 128
    # pad slots duplicate the first used cell; their C columns stay zero
    slot_cols = np.concatenate([cols, np.full(nslots - ncols, cols[0], np.int64)])
    slot_i = slot_cols // T
    slot_t = slot_cols % T

    n_mt = (A + 127) // 128
    Apad = n_mt * 128
    # lhsT layout [slot-in-chunk (K), chunk * Apad + alive-row (M)]
    cw = np.zeros((128, nchunks * Apad), np.float32)
    dw = np.zeros((128, nchunks * Apad), np.float32)
    for s in range(ncols):
        ci = slot_cols[s]
        c, k = divmod(s, 128)
        cw[k, c * Apad : c * Apad + A] = Ca[:, ci]
        dw[k, c * Apad : c * Apad + A] = Ca[:, ci] % 2
    return {
        "A": A,
        "aidx": aidx,
        "slot_i": slot_i,
        "slot_t": slot_t,
        "nchunks": nchunks,
        "n_mt": n_mt,
        "Apad": Apad,
        "cw": cw,
        "dw": dw,
    }


def _build_nc(plan, reps=1, loop=1):
    """reps: python-unrolled body repetitions (for slope benchmarking).
    loop: hardware For_i trip count around the body."""
    f32 = mybir.dt.float32
    bf16 = mybir.dt.bfloat16
    f8 = mybir.dt.float8e4
    u8 = mybir.dt.uint8
    DR = mybir.MatmulPerfMode.DoubleRow
    nchunks, n_mt, Apad = plan["nchunks"], plan["n_mt"], plan["Apad"]
    bfnp = mybir.dt.np(bf16)
    f8np = mybir.dt.np(f8)

    nc = bacc.Bacc("TRN2", debug=False)
    # [slot-in-chunk, chunk, batch], packed on host while sharding (signed p)
    ap_d = nc.dram_tensor("ap_used", [128, nchunks, BL], bf16, kind="ExternalInput")
    # wiring weights: Const tensors inside the NEFF (no per-exec H2D)
    cw_d = nc.inline_tensor(plan["cw"].astype(bfnp), name="cw_const")
    dw_d = nc.inline_tensor(plan["dw"].astype(f8np), name="dw_const")
    out_d = nc.dram_tensor("outs", [Apad, BL], u8, kind="ExternalOutput")

    # split chunks in two halves so ACT starts while the 2nd DMA flies
    h0 = max(1, nchunks // 2)
    halves = [(0, h0), (h0, nchunks)] if nchunks > 1 else [(0, nchunks)]

    with tile.TileContext(nc) as tc, ExitStack() as ctx:
        pool = ctx.enter_context(tc.tile_pool(name="pool", bufs=1))
        tmp = ctx.enter_context(tc.tile_pool(name="tmp", bufs=2))
        mps = ctx.enter_context(tc.tile_pool(name="mps", bufs=1, space="PSUM"))

        cw_s = pool.tile([128, nchunks * Apad], bf16)
        nc.sync.dma_start(cw_s[:], cw_d[:])
        dw_s = pool.tile([128, nchunks * Apad], f8)
        nc.sync.dma_start(dw_s[:], dw_d[:])
        cwv = cw_s.rearrange("k (c a) -> k c a", a=Apad)
        dwv = dw_s.rearrange("k (c a) -> k c a", a=Apad)

        def body():
            for r in range(reps):
                i = r % 2
                apv_d = ap_d.rearrange("k c b -> k (c b)")
                ap_s = pool.tile(
                    [128, nchunks * BL], bf16, name=f"ap{i}", tag=f"ap{i}"
                )
                t_s = pool.tile([128, nchunks * BL], bf16, name=f"t{i}", tag=f"t{i}")
                ta_s = pool.tile(
                    [128, nchunks * BL], bf16, name=f"ta{i}", tag=f"ta{i}"
                )
                v_s = pool.tile([128, nchunks * BL], f8, name=f"v{i}", tag=f"v{i}")
                for c0, c1 in halves:
                    sl = slice(c0 * BL, c1 * BL)
                    nc.sync.dma_start(ap_s[:, sl], apv_d[:, sl])
                    # ---- phase A: tanh (table: exp_and_others) ----
                    nc.scalar.activation(t_s[:, sl], ap_s[:, sl], AF.Tanh, scale=2.0)
                    # signs from raw p on GpSimd (off ACT/DVE critical path)
                    nc.gpsimd.tensor_scalar(
                        v_s[:, sl], ap_s[:, sl], 0.0, None, ALU.is_gt
                    )
                    # |t| on DVE: max(-t, t)
                    nc.vector.scalar_tensor_tensor(
                        ta_s[:, sl], t_s[:, sl], -1.0, t_s[:, sl], ALU.mult, ALU.max
                    )
                vv = v_s.rearrange("k (c b) -> k c b", b=BL)
                tav = ta_s.rearrange("k (c b) -> k c b", b=BL)

                # ---- parity counts: Y = D^T v (fp8 DoubleRow, 2 chunks/pass)
                Y_ps = mps.tile([128, n_mt * BL], f32, name=f"Yp{i}", tag=f"Yp{i}")
                npair = nchunks // 2
                for c in range(npair):
                    for mt in range(n_mt):
                        nc.tensor.matmul(
                            Y_ps[:, mt * BL : (mt + 1) * BL],
                            dwv[:, 2 * c : 2 * c + 2, mt * 128 : (mt + 1) * 128],
                            vv[:, 2 * c : 2 * c + 2, :],
                            start=(c == 0),
                            stop=(c == npair - 1 and nchunks % 2 == 0),
                            perf_mode=DR,
                        )
                if nchunks % 2:  # odd tail chunk: plain fp8 matmul
                    c = nchunks - 1
                    for mt in range(n_mt):
                        nc.tensor.matmul(
                            Y_ps[:, mt * BL : (mt + 1) * BL],
                            dwv[:, c, mt * 128 : (mt + 1) * 128],
                            vv[:, c, :],
                            start=(nchunks == 1),
                            stop=True,
                        )
                # parity: b = [Y odd] - 1/2, exact via fp32 magic round
                g_t = tmp.tile([128, n_mt * BL], f32, tag="p_g")
                nc.vector.tensor_scalar(g_t[:], Y_ps[:], 0.5, MAGIC, ALU.mult, ALU.add)
                h_t = tmp.tile([128, n_mt * BL], f32, tag="p_h")
                nc.vector.tensor_scalar(h_t[:], g_t[:], MAGIC, None, ALU.subtract)
                d_t = tmp.tile([128, n_mt * BL], bf16, tag="p_d")
                nc.vector.scalar_tensor_tensor(
                    d_t[:], Y_ps[:], 0.5, h_t[:], ALU.mult, ALU.subtract
                )
                b_t = pool.tile([128, n_mt * BL], bf16, name=f"b{i}", tag=f"b{i}")
                nc.vector.tensor_scalar(
                    b_t[:], d_t[:], 0.0, 0.5, ALU.not_equal, ALU.subtract
                )

                # ---- phase B: ln + counts matmul + exp (table: nat_log_exp)
                l_s = pool.tile([128, nchunks * BL], bf16, name=f"l{i}", tag=f"l{i}")
                L_ps = mps.tile([128, n_mt * BL], f32, name=f"Lp{i}", tag=f"Lp{i}")
                lv = l_s.rearrange("k (c b) -> k c b", b=BL)
                for c in range(nchunks):
                    nc.scalar.activation(
                        l_s[:, c * BL : (c + 1) * BL], tav[:, c, :], AF.Ln
                    )
                    for mt in range(n_mt):
                        nc.tensor.matmul(
                            L_ps[:, mt * BL : (mt + 1) * BL],
                            cwv[:, c, mt * 128 : (mt + 1) * 128],
                            lv[:, c, :],
                            start=(c == 0),
                            stop=(c == nchunks - 1),
                        )
                e_t = tmp.tile([128, n_mt * BL], bf16, tag="p_e")
                nc.scalar.activation(e_t[:], L_ps[:], AF.Exp)
                y_t = tmp.tile([128, n_mt * BL], bf16, tag="p_y")
                nc.vector.tensor_tensor(y_t[:], b_t[:], e_t[:], ALU.mult)
                q_t = tmp.tile([128, n_mt * BL], u8, tag="p_q")
                nc.vector.tensor_scalar(
                    q_t[:], y_t[:], 254.0, 128.0, ALU.mult, ALU.add
                )
                nc.sync.dma_start(
                    out_d.rearrange("(m k) b -> k m b", m=n_mt),
                    q_t.rearrange("k (m b) -> k m b", m=n_mt),
                )

        if loop > 1:
            with tc.For_i(0, loop):
                body()
        else:
            body()

    nc.compile()
    return nc


_CACHE = {}


def _get_compiled(wa, wb):
    key = (np.asarray(wa).tobytes(), np.asarray(wb).tobytes())
    if key not in _CACHE:
        plan = _build_plan(wa, wb)
        nc = _build_nc(plan) if plan["A"] > 0 else None
        _CACHE[key] = (plan, nc)
    return _CACHE[key]


def make_in_maps(P, plan):
    """Pack per-core input: signed p of used cells, bf16 [128, nchunks, BL]."""
    bf = mybir.dt.np(mybir.dt.bfloat16)
    nchunks = plan["nchunks"]
    sel_all = P[plan["slot_i"], :, plan["slot_t"]]  # [nslots, B] f32
    sel_all = sel_all.reshape(nchunks, 128, B).transpose(1, 0, 2)  # [128,c,B]
    sel_all = sel_all.astype(bf)
    return [
        {"ap_used": np.ascontiguousarray(sel_all[:, :, c * BL : (c + 1) * BL])}
        for c in range(NCORES)
    ]


def run(P, wa, wb, trace=False):
    """Returns (out [B, N_REG] float32, BassKernelResults-or-None)."""
    P = np.asarray(P)
    plan, nc = _get_compiled(wa, wb)
    out = np.full((B, N_REG), 0.5, np.float32)
    if plan["A"] == 0:
        return out, None

    in_maps = make_in_maps(P, plan)
    res = bass_utils.run_bass_kernel_spmd(nc, in_maps, list(range(NCORES)), trace=trace)
    A = plan["A"]
    aidx = plan["aidx"]
    for c in range(NCORES):
        q = np.asarray(res.results[c]["outs"]).astype(np.float32)  # [Apad, BL]
        s_core = (q - 127.75) * (1.0 / 254.0) + 0.5
        out[c * BL : (c + 1) * BL, aidx] = s_core[:A].T
    return out, res


def kernel(P, wa, wb):
    out, _ = run(P, wa, wb, trace=False)
    return out
